# revision 1
# baseline (speedup 1.0000x reference)
"""Multi-head attention (B=2, S=4096, E=512, H=8) on 8 trn2 NeuronCores.

Sharding: 16 (batch, head) pairs -> 2 heads per core (core c: batch c//4,
heads 2*(c%4), 2*(c%4)+1). Each core computes q/k/v projections for its two
heads, full (unscaled-softmax) attention, and a partial output projection
through its rows of Wo. Host sums the 4 partial outputs per batch and adds
the bias terms (bo + bv @ Wo, exact since softmax rows sum to 1).

Device layout notes:
  - activations are fed pre-transposed ([E, S]) so every matmul contracts
    along partitions with fully contiguous DMA.
  - scores are computed transposed ([s_k, s_q] tiles) so exp(scores) tiles
    can be used directly as the stationary operand of the attention matmul.
  - softmax denominators come for free from a ones-column appended to v
    (output column 64 of the attention matmul).
"""

import numpy as np

import concourse.bass as bass
import concourse.mybir as mybir
from concourse.tile import TileContext
from concourse.bass_utils import run_bass_kernel_spmd

B, S, E, H = 2, 4096, 512, 8
DK = E // H  # 64
N_CORES = 8
F32 = mybir.dt.float32
BF16 = mybir.dt.bfloat16
AF = mybir.ActivationFunctionType
ADD = mybir.AluOpType.add
MULT = mybir.AluOpType.mult
DVE_EXP_MOD = 0
EPI_FRONT = False

# ---------------------------------------------------------------------------
# Workaround for walrus "Too many sync wait commands" on the TileContext
# final drain: emit one single-wait SP nop per pending semaphore before the
# drain, and emit the drain itself with no waits.
# ---------------------------------------------------------------------------
import bass_rust

_patched = False


def _split_drain_and_barrier(self, tick_clock, wait_clock):
    gc = tick_clock.global_clock
    counts = eval(repr(gc).replace("VectorClock", ""))
    for proc, cnt in enumerate(counts):
        if cnt <= 0:
            continue
        single = [0] * len(counts)
        single[proc] = cnt
        nop = self.nc.sync.nop(nofuse=True, hint="drain_split")
        wait_clock.add_sem_waits(
            nop.ins, bass_rust.ScopedClock({None: bass_rust.VectorClock(single)})
        )
    self.nc.sync.drain()
    self.nc.all_engine_barrier()
    assert self.sems is not None
    popped = self.nc._tile_sem_poison_stack.pop()
    assert popped is self._sem_poison
    self.nc.clear_and_free_semaphores(list(self.sems.allocated().values()))
    self.nc.all_engine_barrier()


_orig_saa = TileContext.schedule_and_allocate


def _saa_capture(self, *a, **k):
    r = _orig_saa(self, *a, **k)
    try:
        self.predicted_ns = r[1].time if r and r[1] is not None else None
    except Exception:
        self.predicted_ns = None
    return r


def _apply_patch():
    global _patched
    if not _patched:
        TileContext._drain_and_barrier = _split_drain_and_barrier
        TileContext.schedule_and_allocate = _saa_capture
        _patched = True


def _split_multiwait_json(raw: bytes) -> bytes:
    """The walrus build in this container accepts at most ONE sync wait per
    instruction. Hoist extra waits onto single-wait NoOps spliced in just
    before the instruction on the same engine stream (engine streams follow
    block order, so the nops complete before the instruction issues)."""
    import orjson

    j = orjson.loads(raw)
    n_split = 0
    for f in j["functions"]:
        for bb in f["blocks"]:
            out = []
            for inst in bb["instructions"]:
                si = inst.get("sync_info") or {}
                ow = si.get("on_wait") or []
                if len(ow) > 1:
                    for i, w in enumerate(ow[:-1]):
                        out.append(
                            {
                                "name": f"{inst['name']}-wsplit{i}",
                                "opcode": "NoOp",
                                "engine": inst["engine"],
                                "ins": [],
                                "outs": [],
                                "sync_info": {"on_wait": [w], "on_update": []},
                            }
                        )
                        n_split += 1
                    si["on_wait"] = [ow[-1]]
                out.append(inst)
            bb["instructions"] = out
    return orjson.dumps(j)


def _patch_to_json(nc):
    orig = nc.to_json_bytes

    def wrapped(*a, **k):
        return _split_multiwait_json(orig(*a, **k))

    nc.to_json_bytes = wrapped
    return nc


# ---------------------------------------------------------------------------
# Kernel builder (per-core program; SPMD over 8 cores with different data)
# ---------------------------------------------------------------------------


def build(s=S, reps=1):
    """Build the per-core Bass program for sequence length s. reps>1 wraps
    the whole body in an on-device For_i loop (used only for timing).

    Stage A (projections) and stage B (attention) are interleaved in
    program order: the Tile scheduler executes each engine stream roughly
    in program order, so block 0's score/exp work is emitted inside the
    stage-A chunk loop to keep ScalarE busy from the start. Attention
    accumulation is kc-outer (all 8 (head, m) streams accumulate into two
    one-bank psum tiles), so an exp'd score tile is fully consumed right
    after it is produced and the wt pool stays small.
    """
    import contextlib
    from concourse.masks import make_identity

    assert s % 512 == 0
    SC = s // 512  # 512-wide s chunks
    KC = s // 128  # 128-wide kv chunks
    nc = bass.Bass(target_bir_lowering=False, trn_type="TRN2")

    F32R = mybir.dt.float32r
    xq = nc.dram_tensor("xq", [E, s], F32R, kind="ExternalInput")
    xk = nc.dram_tensor("xk", [E, s], F32R, kind="ExternalInput")
    xv = nc.dram_tensor("xv", [E, s], F32, kind="ExternalInput")
    wq = nc.dram_tensor("wq", [E, 128], F32R, kind="ExternalInput")
    wk = nc.dram_tensor("wk", [E, 128], F32R, kind="ExternalInput")
    wv = nc.dram_tensor("wv", [E, 128], F32, kind="ExternalInput")
    wo = nc.dram_tensor("wo", [128, E], F32, kind="ExternalInput")
    bq2 = nc.dram_tensor("bq2", [128, 1], F32, kind="ExternalInput")
    bk2 = nc.dram_tensor("bk2", [128, 1], F32, kind="ExternalInput")
    outT = nc.dram_tensor("outT", [E, s], F32, kind="ExternalOutput")

    with TileContext(nc) as tc:
        with contextlib.ExitStack() as _stack:
            constp = _stack.enter_context(tc.tile_pool(name="const", bufs=1))
            wstage = _stack.enter_context(tc.tile_pool(name="wstage", bufs=2))
            if reps > 1:
                _stack.enter_context(tc.For_i(0, reps, 1))

            # --- weights: wq/wk stay f32r (full-rate fp32 matmul); wv bf16 ---
            wbs = {}
            for name, dram in (("wq", wq), ("wk", wk)):
                rt = constp.tile([128, 512], F32R, tag=f"{name}_r", name=f"{name}_r")
                for ec in range(4):
                    nc.sync.dma_start(
                        rt[:, ec * 128 : (ec + 1) * 128],
                        dram[ec * 128 : (ec + 1) * 128, :],
                    )
                wbs[name] = rt
            f32t = wstage.tile([128, 512], F32, tag="wf")
            for ec in range(4):
                nc.sync.dma_start(
                    f32t[:, ec * 128 : (ec + 1) * 128],
                    wv[ec * 128 : (ec + 1) * 128, :],
                )
            wv_b = constp.tile([128, 512], BF16, tag="wv_b")
            nc.vector.tensor_copy(wv_b[:], f32t[:])
            wbs["wv"] = wv_b
            wo_f = wstage.tile([128, 512], F32, tag="wf")
            nc.sync.dma_start(wo_f[:], wo[:, :])
            wo_b = constp.tile([128, 512], BF16, tag="wo_b")
            nc.vector.tensor_copy(wo_b[:], wo_f[:])

            bq_t = constp.tile([128, 1], F32, tag="bq")
            nc.sync.dma_start(bq_t[:], bq2[:, :])
            bk_t = constp.tile([128, 1], F32, tag="bk")
            nc.sync.dma_start(bk_t[:], bk2[:, :])

            ident = constp.tile([128, 128], BF16, tag="ident")
            make_identity(nc, ident[:])

            # persistent activations, chunked for fine-grained deps
            qTs = [
                constp.tile([128, 512], F32R, tag=f"qT{i}", name=f"qT_{i}")
                for i in range(SC)
            ]
            kTs = [
                constp.tile([128, 512], F32R, tag=f"kT{i}", name=f"kT_{i}")
                for i in range(SC)
            ]
            attnTs = [
                constp.tile([128, 512], BF16, tag=f"attnT{i}", name=f"attnT_{i}")
                for i in range(SC)
            ]
            v2s = [
                constp.tile([128, 4 * 130], BF16, tag=f"v2_{i}", name=f"v2_{i}")
                for i in range(SC)
            ]
            for t in v2s:
                nc.gpsimd.memset(t[:], 1.0)

            pools = _stack.enter_context(contextlib.ExitStack())
            xinp = pools.enter_context(tc.tile_pool(name="xin", bufs=6))
            xbp = pools.enter_context(tc.tile_pool(name="xbp", bufs=6))
            wtp = pools.enter_context(tc.tile_pool(name="wt", bufs=KC + 8))
            schp = pools.enter_context(tc.tile_pool(name="schp", bufs=6))
            miscp = pools.enter_context(tc.tile_pool(name="miscp", bufs=8))
            ostp = pools.enter_context(tc.tile_pool(name="ostp", bufs=3))
            scoresp = pools.enter_context(
                tc.tile_pool(name="scoresp", bufs=2, space="PSUM")
            )
            attnp = pools.enter_context(tc.tile_pool(name="attnp", bufs=2, space="PSUM"))
            projp = pools.enter_context(tc.tile_pool(name="projp", bufs=2, space="PSUM"))

            apsh = {}  # block -> [h0_tile, h1_tile]; each [128, 4*65] (one bank)

            def emit_stage_a_qk(sc):
                for dram, wname, btile, dst in (
                    (xq, "wq", bq_t, qTs[sc]),
                    (xk, "wk", bk_t, kTs[sc]),
                ):
                    ps = projp.tile([128, 512], F32, tag="proj", name=f"ps_{wname}_{sc}")
                    for ec in range(4):
                        xt = xinp.tile([128, 512], F32R, tag="xin", name=f"xt_{wname}_{sc}_{ec}")
                        nc.sync.dma_start(
                            xt[:],
                            dram[ec * 128 : (ec + 1) * 128, sc * 512 : (sc + 1) * 512],
                        )
                        nc.tensor.matmul(
                            ps[:],
                            wbs[wname][:, ec * 128 : (ec + 1) * 128],
                            xt[:],
                            start=(ec == 0),
                            stop=(ec == 3),
                        )
                    nc.vector.tensor_scalar(
                        out=dst[:], in0=ps[:], scalar1=btile[:], scalar2=None, op0=ADD
                    )

            def emit_stage_a_v(sc):
                xvbs = []
                for ec in range(4):
                    xt = xinp.tile([128, 512], F32, tag="xin", name=f"xt_v_{sc}_{ec}")
                    nc.sync.dma_start(
                        xt[:],
                        xv[ec * 128 : (ec + 1) * 128, sc * 512 : (sc + 1) * 512],
                    )
                    xb = xbp.tile([128, 512], BF16, tag="xb", name=f"xb_{sc}_{ec}")
                    nc.vector.tensor_copy(xb[:], xt[:])
                    xvbs.append(xb)
                for mc in range(4):
                    psv = projp.tile([128, 128], F32, tag="proj", name=f"psv_{sc}_{mc}")
                    for ec in range(4):
                        nc.tensor.matmul(
                            psv[:],
                            xvbs[ec][:, mc * 128 : (mc + 1) * 128],
                            wbs["wv"][:, ec * 128 : (ec + 1) * 128],
                            start=(ec == 0),
                            stop=(ec == 3),
                        )
                    nc.vector.tensor_copy(
                        v2s[sc][:, mc * 130 : mc * 130 + 64], psv[:, 0:64]
                    )
                    nc.vector.tensor_copy(
                        v2s[sc][:, mc * 130 + 65 : mc * 130 + 129], psv[:, 64:128]
                    )

            wts = {}
            # exp on DVE via two phase-shifted Schraudolph int32 casts blended
            # with one scalar_tensor_tensor: w = 0.708*sch(x, +0.5oct) + sch(x).
            # Softmax normalization cancels the common scale; residual ripple
            # is ~0.45% rms, below the bf16 storage noise budget.
            SCH_A = 12102203.161561  # log2(e) * 2^23
            # B offsets include -log2(1 + 0.708*2^0.5-ish mean) so the blend's
            # mean scale matches the exact-exp tiles it shares softmax rows with
            SCH_B1 = 1056474367.9
            SCH_B2 = 1060668671.9
            SCH_R = 0.7080
            I32 = mybir.dt.int32
            exp_ctr = [0]

            def emit_scores_exp(sq, kc, allow_dve=False):
                # scores (both heads) -> one 2-bank psum region -> one exp
                ps = scoresp.tile([128, 1024], F32, tag="sc", name=f"ps_{sq}_{kc}")
                kslice = kTs[kc // 4][:, (kc % 4) * 128 : (kc % 4 + 1) * 128]
                for h in (0, 1):
                    nc.tensor.matmul(
                        ps[:, h * 512 : (h + 1) * 512],
                        kslice[h * 64 : (h + 1) * 64, :],
                        qTs[sq][h * 64 : (h + 1) * 64, :],
                        start=True,
                        stop=True,
                    )
                wt = wtp.tile([128, 1024], BF16, tag="wt", name=f"wt_{sq}_{kc}")
                exp_ctr[0] += 1
                if allow_dve and DVE_EXP_MOD and exp_ctr[0] % DVE_EXP_MOD == 0:
                    i1 = schp.tile([128, 1024], I32, tag="sch", name=f"s1_{sq}_{kc}")
                    nc.vector.tensor_scalar(
                        out=i1[:], in0=ps[:], scalar1=SCH_A, scalar2=SCH_B1,
                        op0=MULT, op1=ADD,
                    )
                    i2 = schp.tile([128, 1024], I32, tag="sch", name=f"s2_{sq}_{kc}")
                    # exact int shift: round(x*A+B2) == round(x*A+B1) + 0.5*2^23
                    nc.vector.tensor_scalar(
                        out=i2[:], in0=i1[:], scalar1=4194304, scalar2=None,
                        op0=ADD,
                    )
                    nc.vector.scalar_tensor_tensor(
                        out=wt[:], in0=i2[:].bitcast(F32), scalar=SCH_R,
                        in1=i1[:].bitcast(F32), op0=MULT, op1=ADD,
                    )
                else:
                    nc.scalar.activation(wt[:], ps[:], AF.Exp)
                wts[(sq, kc)] = wt

            def emit_attn(sq, kc):
                wt = wts.pop((sq, kc))
                if sq not in apsh:
                    apsh[sq] = [
                        attnp.tile([128, 260], F32, tag="at", name=f"aps_{sq}_{h}")
                        for h in (0, 1)
                    ]
                for h in (0, 1):
                    for m in range(4):
                        # start=True clears has_written for the whole psum
                        # bank, so only the first matmul into each one-bank
                        # accumulator tile may set it; the other slices'
                        # first writes overwrite via cleared has_written.
                        nc.tensor.matmul(
                            apsh[sq][h][:, m * 65 : (m + 1) * 65],
                            wt[:, h * 512 + m * 128 : h * 512 + (m + 1) * 128],
                            v2s[kc // 4][
                                :,
                                (kc % 4) * 130 + h * 65 : (kc % 4) * 130 + (h + 1) * 65,
                            ],
                            start=(kc == 0 and m == 0),
                            stop=(kc == KC - 1 and m == 3),
                            skip_group_check=True,
                        )

            def emit_block_epilogue(sq):
                pairs = []
                for m in range(4):
                    pairs.append(
                        miscp.tile([128, 128], BF16, tag="pair", name=f"pair_{sq}_{m}")
                    )
                for h in (0, 1):
                    for m in range(4):
                        rcp = miscp.tile([128, 1], F32, tag="rcp", name=f"rcp_{sq}_{h}_{m}")
                        nc.vector.reciprocal(
                            rcp[:], apsh[sq][h][:, m * 65 + 64 : m * 65 + 65]
                        )
                        nc.vector.tensor_scalar(
                            out=pairs[m][:, h * 64 : (h + 1) * 64],
                            in0=apsh[sq][h][:, m * 65 : m * 65 + 64],
                            scalar1=rcp[:],
                            scalar2=None,
                            op0=MULT,
                        )
                del apsh[sq]
                for m in range(4):
                    tp = projp.tile([128, 128], BF16, tag="proj", name=f"tp_{sq}_{m}")
                    nc.tensor.transpose(tp[:], pairs[m][:], ident[:])
                    nc.vector.tensor_copy(attnTs[sq][:, m * 128 : (m + 1) * 128], tp[:])
                for oc in range(4):
                    po = projp.tile([128, 512], F32, tag="proj", name=f"po_{sq}_{oc}")
                    nc.tensor.matmul(
                        po[:],
                        wo_b[:, oc * 128 : (oc + 1) * 128],
                        attnTs[sq][:],
                        start=True,
                        stop=True,
                    )
                    ost = ostp.tile([128, 512], F32, tag="ost", name=f"ost_{sq}_{oc}")
                    nc.vector.tensor_copy(ost[:], po[:])
                    nc.sync.dma_start(
                        outT[oc * 128 : (oc + 1) * 128, sq * 512 : (sq + 1) * 512],
                        ost[:],
                    )

            # phase 1: stage A chunks interleaved with block 0 scores+attn
            # and block 1 scores (lagged one chunk) to keep ScalarE fed
            for sc in range(SC):
                emit_stage_a_qk(sc)
                for kc in range(4 * sc, 4 * sc + 4):
                    emit_scores_exp(0, kc)
                emit_stage_a_v(sc)
                for kc in range(4 * sc, 4 * sc + 4):
                    emit_attn(0, kc)
                if sc >= 1 and SC > 1:
                    for kc in range(4 * (sc - 1), 4 * sc):
                        emit_scores_exp(1, kc)
            if SC > 1:
                for kc in range(4 * (SC - 1), KC):
                    emit_scores_exp(1, kc)
            # phase 2: per block, stay one block ahead on scores so ScalarE
            # never starves during attention/epilogue work
            for sq in range(1, SC):
                if EPI_FRONT:
                    emit_block_epilogue(sq - 1)
                for kc in range(KC):
                    if sq + 1 < SC:
                        emit_scores_exp(sq + 1, kc, allow_dve=True)
                    emit_attn(sq, kc)
                if not EPI_FRONT:
                    emit_block_epilogue(sq - 1)
            emit_block_epilogue(SC - 1)
    nc._predicted_ns = getattr(tc, "predicted_ns", None)
    return _patch_to_json(nc)


# ---------------------------------------------------------------------------
# Host-side sharding / gathering
# ---------------------------------------------------------------------------


def make_in_maps(query, key_in, value, Wq, bq, Wk, bk, Wv, bv, Wo, bo, s=S):
    in_maps = []
    for c in range(N_CORES):
        b = c // 4
        hs = (c % 4) * 2 * DK  # column offset of this core's two heads
        in_maps.append(
            {
                "xq": np.ascontiguousarray(query[b, :s].T),
                "xk": np.ascontiguousarray(key_in[b, :s].T),
                "xv": np.ascontiguousarray(value[b, :s].T),
                "wq": np.ascontiguousarray(Wq[:, hs : hs + 128]),
                "wk": np.ascontiguousarray(Wk[:, hs : hs + 128]),
                "wv": np.ascontiguousarray(Wv[:, hs : hs + 128]),
                "wo": np.ascontiguousarray(Wo[hs : hs + 128, :]),
                "bq2": np.ascontiguousarray(bq[hs : hs + 128, None]),
                "bk2": np.ascontiguousarray(bk[hs : hs + 128, None]),
            }
        )
    return in_maps


def assemble(results, bv, Wo, bo, s=S):
    out = np.zeros((B, s, E), np.float32)
    for c in range(N_CORES):
        out[c // 4] += results[c]["outT"].T
    out += (bo + bv @ Wo)[None, None, :]
    return out


_nc_cache = {}


def kernel(query, key_in, value, Wq, bq, Wk, bk, Wv, bv, Wo, bo):
    _apply_patch()
    query = np.asarray(query, np.float32)
    key_in = np.asarray(key_in, np.float32)
    value = np.asarray(value, np.float32)
    Wq, bq = np.asarray(Wq, np.float32), np.asarray(bq, np.float32)
    Wk, bk = np.asarray(Wk, np.float32), np.asarray(bk, np.float32)
    Wv, bv = np.asarray(Wv, np.float32), np.asarray(bv, np.float32)
    Wo, bo = np.asarray(Wo, np.float32), np.asarray(bo, np.float32)

    if S not in _nc_cache:
        _nc_cache[S] = build(S)
    nc = _nc_cache[S]
    in_maps = make_in_maps(query, key_in, value, Wq, bq, Wk, bk, Wv, bv, Wo, bo)
    res = run_bass_kernel_spmd(nc, in_maps, core_ids=list(range(N_CORES)))
    return assemble(res.results, bv, Wo, bo)



# revision 8
# speedup vs baseline: 1.2146x; 1.2146x over previous
"""Multi-head attention (B=2, S=4096, E=512, H=8) on 8 trn2 NeuronCores.

Sharding: 16 (batch, head) pairs -> 2 heads per core (core c: batch c//4,
heads 2*(c%4), 2*(c%4)+1). Each core computes q/k/v projections for its two
heads, full (unscaled-softmax) attention, and a partial output projection
through its rows of Wo. Host sums the 4 partial outputs per batch and adds
the bias terms (bo + bv @ Wo, exact since softmax rows sum to 1).

Device layout notes:
  - activations are fed pre-transposed ([E, S]) so every matmul contracts
    along partitions with fully contiguous DMA.
  - scores are computed transposed ([s_k, s_q] tiles) so exp(scores) tiles
    can be used directly as the stationary operand of the attention matmul.
  - softmax denominators come for free from a ones-column appended to v
    (output column 64 of the attention matmul).
"""

import numpy as np

import concourse.bass as bass
import concourse.mybir as mybir
from concourse.tile import TileContext
from concourse.bass_utils import run_bass_kernel_spmd

B, S, E, H = 2, 4096, 512, 8
DK = E // H  # 64
N_CORES = 8
F32 = mybir.dt.float32
BF16 = mybir.dt.bfloat16
AF = mybir.ActivationFunctionType
ADD = mybir.AluOpType.add
MULT = mybir.AluOpType.mult
DVE_EXP_MOD = 3
EPI_FRONT = False

# ---------------------------------------------------------------------------
# Workaround for walrus "Too many sync wait commands" on the TileContext
# final drain: emit one single-wait SP nop per pending semaphore before the
# drain, and emit the drain itself with no waits.
# ---------------------------------------------------------------------------
import bass_rust

_patched = False


def _split_drain_and_barrier(self, tick_clock, wait_clock):
    gc = tick_clock.global_clock
    counts = eval(repr(gc).replace("VectorClock", ""))
    for proc, cnt in enumerate(counts):
        if cnt <= 0:
            continue
        single = [0] * len(counts)
        single[proc] = cnt
        nop = self.nc.sync.nop(nofuse=True, hint="drain_split")
        wait_clock.add_sem_waits(
            nop.ins, bass_rust.ScopedClock({None: bass_rust.VectorClock(single)})
        )
    self.nc.sync.drain()
    self.nc.all_engine_barrier()
    assert self.sems is not None
    popped = self.nc._tile_sem_poison_stack.pop()
    assert popped is self._sem_poison
    self.nc.clear_and_free_semaphores(list(self.sems.allocated().values()))
    self.nc.all_engine_barrier()


_orig_saa = TileContext.schedule_and_allocate


def _saa_capture(self, *a, **k):
    r = _orig_saa(self, *a, **k)
    try:
        self.predicted_ns = r[1].time if r and r[1] is not None else None
    except Exception:
        self.predicted_ns = None
    return r


def _apply_patch():
    global _patched
    if not _patched:
        TileContext._drain_and_barrier = _split_drain_and_barrier
        TileContext.schedule_and_allocate = _saa_capture
        _patched = True


def _split_multiwait_json(raw: bytes) -> bytes:
    """The walrus build in this container accepts at most ONE sync wait per
    instruction. Hoist extra waits onto single-wait NoOps spliced in just
    before the instruction on the same engine stream (engine streams follow
    block order, so the nops complete before the instruction issues)."""
    import orjson

    j = orjson.loads(raw)
    n_split = 0
    for f in j["functions"]:
        for bb in f["blocks"]:
            out = []
            for inst in bb["instructions"]:
                si = inst.get("sync_info") or {}
                ow = si.get("on_wait") or []
                if len(ow) > 1:
                    for i, w in enumerate(ow[:-1]):
                        out.append(
                            {
                                "name": f"{inst['name']}-wsplit{i}",
                                "opcode": "NoOp",
                                "engine": inst["engine"],
                                "ins": [],
                                "outs": [],
                                "sync_info": {"on_wait": [w], "on_update": []},
                            }
                        )
                        n_split += 1
                    si["on_wait"] = [ow[-1]]
                out.append(inst)
            bb["instructions"] = out
    return orjson.dumps(j)


def _patch_to_json(nc):
    orig = nc.to_json_bytes

    def wrapped(*a, **k):
        return _split_multiwait_json(orig(*a, **k))

    nc.to_json_bytes = wrapped
    return nc


# ---------------------------------------------------------------------------
# Kernel builder (per-core program; SPMD over 8 cores with different data)
# ---------------------------------------------------------------------------


def build(s=S, reps=1):
    """Build the per-core Bass program for sequence length s. reps>1 wraps
    the whole body in an on-device For_i loop (used only for timing).

    v2 pipeline:
      - exp work (the co-bottleneck) is greedily load-balanced between
        ScalarE (exact Exp) and DVE (single-instruction Schraudolph in
        bf16 bit-space); all other PSUM evacuations are likewise greedily
        split between the two engines. GPSIMD does SBUF->SBUF bf16 casts.
      - scores psum is triple-buffered (6 of 8 banks) so the PE->exp->PE
        buffer-recycle latency loop no longer gates throughput; stage-A
        projection / epilogue psum tiles share the scores slots.
      - q projections for blocks 2.. are deferred into phase 2 (one per
        block) so input DMA (~360GB/s/core aggregate, the phase-1 floor)
        spreads across the whole kernel.
      - input/output DMAs are batched 4 row-chunks at a time via
        rearranged access patterns.
    """
    import contextlib
    from concourse.masks import make_identity

    assert s % 512 == 0
    SC = s // 512  # 512-wide s chunks
    KC = s // 128  # 128-wide kv chunks
    nc = bass.Bass(target_bir_lowering=False, trn_type="TRN2")

    F32R = mybir.dt.float32r
    xq = nc.dram_tensor("xq", [E, s], F32R, kind="ExternalInput")
    xk = nc.dram_tensor("xk", [E, s], F32R, kind="ExternalInput")
    xv = nc.dram_tensor("xv", [E, s], F32, kind="ExternalInput")
    wq = nc.dram_tensor("wq", [E, 128], F32R, kind="ExternalInput")
    wk = nc.dram_tensor("wk", [E, 128], F32R, kind="ExternalInput")
    wv = nc.dram_tensor("wv", [E, 128], F32, kind="ExternalInput")
    wo = nc.dram_tensor("wo", [128, E], F32, kind="ExternalInput")
    bq2 = nc.dram_tensor("bq2", [128, 1], F32, kind="ExternalInput")
    bk2 = nc.dram_tensor("bk2", [128, 1], F32, kind="ExternalInput")
    outT = nc.dram_tensor("outT", [E, s], F32, kind="ExternalOutput")

    def quad(dram_ap):
        # [512, c] dram slice -> [128, 4, c] AP (4 row-chunks stacked)
        return dram_ap.rearrange("(e p) c -> p e c", e=4)

    def sb4(sbuf_ap):
        # [128, 4*c] sbuf AP -> [128, 4, c] to pair with quad()
        return sbuf_ap.rearrange("p (e c) -> p e c", e=4)

    with TileContext(nc) as tc:
        with contextlib.ExitStack() as _stack:
            constp = _stack.enter_context(tc.tile_pool(name="const", bufs=1))
            if reps > 1:
                _stack.enter_context(tc.For_i(0, reps, 1))

            # ---- engine load balancing (ScalarE vs DVE) for PSUM work ----
            eng_load = {"A": 0.0, "D": 0.0}

            def pick_engine(a_cost, d_cost):
                if eng_load["A"] + a_cost <= eng_load["D"] + d_cost:
                    eng_load["A"] += a_cost
                    return "A"
                eng_load["D"] += d_cost
                return "D"

            def evac(dst, src, bias=None, scale=None, n=None):
                # PSUM -> SBUF move (opt. +bias or *scale) on the lighter engine
                if n is None:
                    n = src.shape[-1] if len(src.shape) == 2 else src.free_size()
                e = pick_engine(n * 0.833 + 330, n * 1.042 + 250)
                if e == "A":
                    nc.scalar.activation(
                        dst, src,
                        AF.Identity if bias is not None else AF.Copy,
                        bias=bias if bias is not None else 0.0,
                        scale=scale if scale is not None else 1.0,
                    )
                elif bias is not None:
                    nc.vector.tensor_scalar(
                        out=dst, in0=src, scalar1=bias, scalar2=None, op0=ADD
                    )
                elif scale is not None:
                    nc.vector.tensor_scalar(
                        out=dst, in0=src, scalar1=scale, scalar2=None, op0=MULT
                    )
                else:
                    nc.vector.tensor_copy(dst, src)

            # --- weights: wq/wk stay f32r (full-rate fp32 matmul); wv bf16 ---
            wbs = {}
            for name, dram in (("wq", wq), ("wk", wk)):
                rt = constp.tile([128, 512], F32R, tag=f"{name}_r", name=f"{name}_r")
                nc.sync.dma_start(sb4(rt[:]), quad(dram[:, :]))
                wbs[name] = rt
            wstage = _stack.enter_context(tc.tile_pool(name="wstage", bufs=2))
            f32t = wstage.tile([128, 512], F32, tag="wf")
            nc.sync.dma_start(sb4(f32t[:]), quad(wv[:, :]))
            wv_b = constp.tile([128, 512], BF16, tag="wv_b")
            nc.gpsimd.tensor_copy(wv_b[:], f32t[:])
            wbs["wv"] = wv_b
            wo_f = wstage.tile([128, 512], F32, tag="wf")
            nc.sync.dma_start(wo_f[:], wo[:, :])
            wo_b = constp.tile([128, 512], BF16, tag="wo_b")
            nc.gpsimd.tensor_copy(wo_b[:], wo_f[:])

            bq_t = constp.tile([128, 1], F32, tag="bq")
            nc.sync.dma_start(bq_t[:], bq2[:, :])
            bk_t = constp.tile([128, 1], F32, tag="bk")
            nc.sync.dma_start(bk_t[:], bk2[:, :])

            ident = constp.tile([128, 128], BF16, tag="ident")
            make_identity(nc, ident[:])

            # persistent activations, chunked for fine-grained deps
            qTs = [
                constp.tile([128, 512], F32R, tag=f"qT{i}", name=f"qT_{i}")
                for i in range(SC)
            ]
            kTs = [
                constp.tile([128, 512], F32R, tag=f"kT{i}", name=f"kT_{i}")
                for i in range(SC)
            ]
            attnTs = [
                constp.tile([128, 512], BF16, tag=f"attnT{i}", name=f"attnT_{i}")
                for i in range(SC)
            ]
            v2s = [
                constp.tile([128, 4 * 130], BF16, tag=f"v2_{i}", name=f"v2_{i}")
                for i in range(SC)
            ]
            for t in v2s:
                nc.gpsimd.memset(t[:], 1.0)

            pools = _stack.enter_context(contextlib.ExitStack())
            xinp = pools.enter_context(tc.tile_pool(name="xin", bufs=4))
            xbp = pools.enter_context(tc.tile_pool(name="xbp", bufs=2))
            wtp = pools.enter_context(tc.tile_pool(name="wt", bufs=KC + 8))
            miscp = pools.enter_context(tc.tile_pool(name="miscp", bufs=8))
            ostp = pools.enter_context(tc.tile_pool(name="ostp", bufs=2))
            scoresp = pools.enter_context(
                tc.tile_pool(name="scoresp", bufs=3, space="PSUM")
            )
            attnp = pools.enter_context(tc.tile_pool(name="attnp", bufs=2, space="PSUM"))

            apsh = {}  # block -> [h0_tile, h1_tile]; each [128, 4*65] (one bank)

            def emit_qk_proj(sc, which):
                dram, wname, btile, dst = (
                    (xq, "wq", bq_t, qTs[sc]) if which == "q" else (xk, "wk", bk_t, kTs[sc])
                )
                xt = xinp.tile([128, 2048], F32R, tag="xin", name=f"xt_{wname}_{sc}")
                nc.sync.dma_start(sb4(xt[:]), quad(dram[:, sc * 512 : (sc + 1) * 512]))
                ps = scoresp.tile([128, 512], F32, tag="sc", name=f"ps_{wname}_{sc}")
                for ec in range(4):
                    nc.tensor.matmul(
                        ps[:],
                        wbs[wname][:, ec * 128 : (ec + 1) * 128],
                        xt[:, ec * 512 : (ec + 1) * 512],
                        start=(ec == 0),
                        stop=(ec == 3),
                    )
                evac(dst[:], ps[:], bias=btile[:])

            def emit_v_proj(sc):
                xt = xinp.tile([128, 2048], F32, tag="xin", name=f"xt_v_{sc}")
                nc.sync.dma_start(sb4(xt[:]), quad(xv[:, sc * 512 : (sc + 1) * 512]))
                xb = xbp.tile([128, 2048], BF16, tag="xb", name=f"xb_{sc}")
                nc.gpsimd.tensor_copy(xb[:], xt[:])
                v_ps = scoresp.tile([128, 512], F32, tag="sc", name=f"psv_{sc}")
                first = True
                for mc in range(4):
                    for ec in range(4):
                        nc.tensor.matmul(
                            v_ps[:, mc * 128 : (mc + 1) * 128],
                            xb[:, ec * 512 + mc * 128 : ec * 512 + (mc + 1) * 128],
                            wbs["wv"][:, ec * 128 : (ec + 1) * 128],
                            start=first,
                            stop=(mc == 3 and ec == 3),
                            skip_group_check=not first,
                        )
                        first = False
                # evac [128, (4 mc), 64] halves into the 65-strided v2 layout
                src3 = v_ps[:].rearrange("p (m z) -> p m z", m=4)
                dst3 = v2s[sc][:].rearrange("p (m z) -> p m z", m=4)
                for h in (0, 1):
                    evac(
                        dst3[:, :, h * 65 : h * 65 + 64],
                        src3[:, :, h * 64 : (h + 1) * 64],
                        n=256,
                    )

            wts = {}
            # DVE exp: single-instruction Schraudolph in bf16 bit-space.
            # i16 = round(x * 2^7/ln2 + (127*2^7 - c)) IS the bf16 bit pattern
            # of ~e^x (linear mantissa interp). c=7 zeroes the mean log-scale
            # mismatch vs the exact-exp tiles sharing softmax rows; residual
            # ripple ~1.8% rms on offloaded tiles only.
            SCHB_A = 184.6650171  # 2^7 / ln(2)
            SCHB_B = 127.0 * 128.0 - 7.0
            I16 = mybir.dt.int16

            def emit_scores_exp(sq, kc):
                # scores (both heads) -> one 2-bank psum region -> one exp
                ps = scoresp.tile([128, 1024], F32, tag="sc", name=f"ps_{sq}_{kc}")
                kslice = kTs[kc // 4][:, (kc % 4) * 128 : (kc % 4 + 1) * 128]
                for h in (0, 1):
                    nc.tensor.matmul(
                        ps[:, h * 512 : (h + 1) * 512],
                        kslice[h * 64 : (h + 1) * 64, :],
                        qTs[sq][h * 64 : (h + 1) * 64, :],
                        start=True,
                        stop=True,
                    )
                wt = wtp.tile([128, 1024], BF16, tag="wt", name=f"wt_{sq}_{kc}")
                e = pick_engine(1024 * 0.833 + 330, 1024 * 1.042 + 250)
                if e == "D":
                    nc.vector.tensor_scalar(
                        out=wt[:].bitcast(I16), in0=ps[:], scalar1=SCHB_A,
                        scalar2=SCHB_B, op0=MULT, op1=ADD,
                    )
                else:
                    nc.scalar.activation(wt[:], ps[:], AF.Exp)
                wts[(sq, kc)] = wt

            def emit_attn(sq, kc):
                wt = wts.pop((sq, kc))
                if sq not in apsh:
                    apsh[sq] = [
                        attnp.tile([128, 260], F32, tag="at", name=f"aps_{sq}_{h}")
                        for h in (0, 1)
                    ]
                for h in (0, 1):
                    for m in range(4):
                        # start=True clears has_written for the whole psum
                        # bank, so only the first matmul into each one-bank
                        # accumulator tile may set it; the other slices'
                        # first writes overwrite via cleared has_written.
                        nc.tensor.matmul(
                            apsh[sq][h][:, m * 65 : (m + 1) * 65],
                            wt[:, h * 512 + m * 128 : h * 512 + (m + 1) * 128],
                            v2s[kc // 4][
                                :,
                                (kc % 4) * 130 + h * 65 : (kc % 4) * 130 + (h + 1) * 65,
                            ],
                            start=(kc == 0 and m == 0),
                            stop=(kc == KC - 1 and m == 3),
                            skip_group_check=True,
                        )

            def emit_block_epilogue(sq):
                pairs = []
                for m in range(4):
                    pairs.append(
                        miscp.tile([128, 128], BF16, tag="pair", name=f"pair_{sq}_{m}")
                    )
                for h in (0, 1):
                    rcp = miscp.tile([128, 4], F32, tag="rcp", name=f"rcp_{sq}_{h}")
                    den = apsh[sq][h][:].rearrange("p (m z) -> p m z", m=4)
                    nc.vector.reciprocal(rcp[:], den[:, :, 64:65].squeeze(2))
                    for m in range(4):
                        evac(
                            pairs[m][:, h * 64 : (h + 1) * 64],
                            apsh[sq][h][:, m * 65 : m * 65 + 64],
                            scale=rcp[:, m : m + 1],
                        )
                del apsh[sq]
                ost = ostp.tile([128, 2048], F32, tag="ost", name=f"ost_{sq}")
                for m in range(4):
                    tp = scoresp.tile([128, 128], BF16, tag="sc", name=f"tp_{sq}_{m}")
                    nc.tensor.transpose(tp[:], pairs[m][:], ident[:])
                    evac(attnTs[sq][:, m * 128 : (m + 1) * 128], tp[:])
                for oc in range(4):
                    po = scoresp.tile([128, 512], F32, tag="sc", name=f"po_{sq}_{oc}")
                    nc.tensor.matmul(
                        po[:],
                        wo_b[:, oc * 128 : (oc + 1) * 128],
                        attnTs[sq][:],
                        start=True,
                        stop=True,
                    )
                    evac(ost[:, oc * 512 : (oc + 1) * 512], po[:])
                nc.sync.dma_start(
                    quad(outT[:, sq * 512 : (sq + 1) * 512]), sb4(ost[:])
                )

            # phase 1: k/v projections + q(0,1), interleaved with block-0
            # scores+attn and block-1 scores (lagged one chunk)
            for sc in range(SC):
                emit_qk_proj(sc, "k")
                if sc <= 1:
                    emit_qk_proj(sc, "q")
                emit_v_proj(sc)
                for kc in range(4 * sc, 4 * sc + 4):
                    emit_scores_exp(0, kc)
                for kc in range(4 * sc, 4 * sc + 4):
                    emit_attn(0, kc)
                if sc >= 1 and SC > 1:
                    for kc in range(4 * (sc - 1), 4 * sc):
                        emit_scores_exp(1, kc)
            if SC > 1:
                for kc in range(4 * (SC - 1), KC):
                    emit_scores_exp(1, kc)
            # phase 2: per block, stay one block ahead on scores; epilogue
            # of the previous block runs up front so its apsh psum banks
            # free early for this block's accumulation
            for sq in range(1, SC):
                if sq + 1 < SC:
                    emit_qk_proj(sq + 1, "q")
                    emit_scores_exp(sq + 1, 0)
                    emit_scores_exp(sq + 1, 1)
                emit_block_epilogue(sq - 1)
                for kc in range(KC):
                    if sq + 1 < SC and kc >= 2:
                        emit_scores_exp(sq + 1, kc)
                    emit_attn(sq, kc)
                if sq + 1 < SC:
                    pass
            emit_block_epilogue(SC - 1)
    nc._predicted_ns = getattr(tc, "predicted_ns", None)
    return _patch_to_json(nc)


# ---------------------------------------------------------------------------
# Host-side sharding / gathering
# ---------------------------------------------------------------------------


def make_in_maps(query, key_in, value, Wq, bq, Wk, bk, Wv, bv, Wo, bo, s=S):
    in_maps = []
    for c in range(N_CORES):
        b = c // 4
        hs = (c % 4) * 2 * DK  # column offset of this core's two heads
        in_maps.append(
            {
                "xq": np.ascontiguousarray(query[b, :s].T),
                "xk": np.ascontiguousarray(key_in[b, :s].T),
                "xv": np.ascontiguousarray(value[b, :s].T),
                "wq": np.ascontiguousarray(Wq[:, hs : hs + 128]),
                "wk": np.ascontiguousarray(Wk[:, hs : hs + 128]),
                "wv": np.ascontiguousarray(Wv[:, hs : hs + 128]),
                "wo": np.ascontiguousarray(Wo[hs : hs + 128, :]),
                "bq2": np.ascontiguousarray(bq[hs : hs + 128, None]),
                "bk2": np.ascontiguousarray(bk[hs : hs + 128, None]),
            }
        )
    return in_maps


def assemble(results, bv, Wo, bo, s=S):
    out = np.zeros((B, s, E), np.float32)
    for c in range(N_CORES):
        out[c // 4] += results[c]["outT"].T
    out += (bo + bv @ Wo)[None, None, :]
    return out


_nc_cache = {}


def kernel(query, key_in, value, Wq, bq, Wk, bk, Wv, bv, Wo, bo):
    _apply_patch()
    query = np.asarray(query, np.float32)
    key_in = np.asarray(key_in, np.float32)
    value = np.asarray(value, np.float32)
    Wq, bq = np.asarray(Wq, np.float32), np.asarray(bq, np.float32)
    Wk, bk = np.asarray(Wk, np.float32), np.asarray(bk, np.float32)
    Wv, bv = np.asarray(Wv, np.float32), np.asarray(bv, np.float32)
    Wo, bo = np.asarray(Wo, np.float32), np.asarray(bo, np.float32)

    if S not in _nc_cache:
        _nc_cache[S] = build(S)
    nc = _nc_cache[S]
    in_maps = make_in_maps(query, key_in, value, Wq, bq, Wk, bk, Wv, bv, Wo, bo)
    res = run_bass_kernel_spmd(nc, in_maps, core_ids=list(range(N_CORES)))
    return assemble(res.results, bv, Wo, bo)



# revision 17
# speedup vs baseline: 1.3928x; 1.1467x over previous
"""Multi-head attention (B=2, S=4096, E=512, H=8) on 8 trn2 NeuronCores.

Sharding: 16 (batch, head) pairs -> 2 heads per core (core c: batch c//4,
heads 2*(c%4), 2*(c%4)+1). Each core computes q/k/v projections for its two
heads, full (unscaled-softmax) attention, and a partial output projection
through its rows of Wo. Host sums the 4 partial outputs per batch and adds
the bias terms (bo + bv @ Wo, exact since softmax rows sum to 1).

Device layout notes:
  - activations are fed pre-transposed ([E, S]) so every matmul contracts
    along partitions with fully contiguous DMA.
  - scores are computed transposed ([s_k, s_q] tiles) so exp(scores) tiles
    can be used directly as the stationary operand of the attention matmul.
  - softmax denominators come for free from a ones-column appended to v
    (output column 64 of the attention matmul).
"""

import numpy as np

import concourse.bass as bass
import concourse.mybir as mybir
from concourse.tile import TileContext
from concourse.bass_utils import run_bass_kernel_spmd

B, S, E, H = 2, 4096, 512, 8
DK = E // H  # 64
N_CORES = 8
F32 = mybir.dt.float32
BF16 = mybir.dt.bfloat16
AF = mybir.ActivationFunctionType
ADD = mybir.AluOpType.add
MULT = mybir.AluOpType.mult
DVE_EXP_MOD = 3
EPI_FRONT = False

# ---------------------------------------------------------------------------
# Workaround for walrus "Too many sync wait commands" on the TileContext
# final drain: emit one single-wait SP nop per pending semaphore before the
# drain, and emit the drain itself with no waits.
# ---------------------------------------------------------------------------
import bass_rust

_patched = False


def _split_drain_and_barrier(self, tick_clock, wait_clock):
    gc = tick_clock.global_clock
    counts = eval(repr(gc).replace("VectorClock", ""))
    for proc, cnt in enumerate(counts):
        if cnt <= 0:
            continue
        single = [0] * len(counts)
        single[proc] = cnt
        nop = self.nc.sync.nop(nofuse=True, hint="drain_split")
        wait_clock.add_sem_waits(
            nop.ins, bass_rust.ScopedClock({None: bass_rust.VectorClock(single)})
        )
    self.nc.sync.drain()
    self.nc.all_engine_barrier()
    assert self.sems is not None
    popped = self.nc._tile_sem_poison_stack.pop()
    assert popped is self._sem_poison
    self.nc.clear_and_free_semaphores(list(self.sems.allocated().values()))
    self.nc.all_engine_barrier()


_orig_saa = TileContext.schedule_and_allocate


def _saa_capture(self, *a, **k):
    r = _orig_saa(self, *a, **k)
    try:
        self.predicted_ns = r[1].time if r and r[1] is not None else None
    except Exception:
        self.predicted_ns = None
    return r


def _apply_patch():
    global _patched
    if not _patched:
        TileContext._drain_and_barrier = _split_drain_and_barrier
        TileContext.schedule_and_allocate = _saa_capture
        _patched = True


def _split_multiwait_json(raw: bytes) -> bytes:
    """The walrus build in this container accepts at most ONE sync wait per
    instruction. Hoist extra waits onto single-wait NoOps spliced in just
    before the instruction on the same engine stream (engine streams follow
    block order, so the nops complete before the instruction issues)."""
    import orjson

    j = orjson.loads(raw)
    n_split = 0
    for f in j["functions"]:
        for bb in f["blocks"]:
            out = []
            for inst in bb["instructions"]:
                si = inst.get("sync_info") or {}
                ow = si.get("on_wait") or []
                if len(ow) > 1:
                    for i, w in enumerate(ow[:-1]):
                        out.append(
                            {
                                "name": f"{inst['name']}-wsplit{i}",
                                "opcode": "NoOp",
                                "engine": inst["engine"],
                                "ins": [],
                                "outs": [],
                                "sync_info": {"on_wait": [w], "on_update": []},
                            }
                        )
                        n_split += 1
                    si["on_wait"] = [ow[-1]]
                out.append(inst)
            bb["instructions"] = out
    return orjson.dumps(j)


def _patch_to_json(nc):
    orig = nc.to_json_bytes

    def wrapped(*a, **k):
        return _split_multiwait_json(orig(*a, **k))

    nc.to_json_bytes = wrapped
    return nc


# ---------------------------------------------------------------------------
# Kernel builder (per-core program; SPMD over 8 cores with different data)
# ---------------------------------------------------------------------------


def build(s=S, reps=1):
    """Build the per-core Bass program for sequence length s. reps>1 wraps
    the whole body in an on-device For_i loop (used only for timing).

    v2 pipeline:
      - exp work (the co-bottleneck) is greedily load-balanced between
        ScalarE (exact Exp) and DVE (single-instruction Schraudolph in
        bf16 bit-space); all other PSUM evacuations are likewise greedily
        split between the two engines. GPSIMD does SBUF->SBUF bf16 casts.
      - scores psum is triple-buffered (6 of 8 banks) so the PE->exp->PE
        buffer-recycle latency loop no longer gates throughput; stage-A
        projection / epilogue psum tiles share the scores slots.
      - q projections for blocks 2.. are deferred into phase 2 (one per
        block) so input DMA (~360GB/s/core aggregate, the phase-1 floor)
        spreads across the whole kernel.
      - input/output DMAs are batched 4 row-chunks at a time via
        rearranged access patterns.
    """
    import contextlib
    from concourse.masks import make_identity

    assert s % 512 == 0
    SC = s // 512  # 512-wide s chunks
    KC = s // 128  # 128-wide kv chunks
    nc = bass.Bass(target_bir_lowering=False, trn_type="TRN2")

    F32R = mybir.dt.float32r
    xq = nc.dram_tensor("xq", [E, s], BF16, kind="ExternalInput")
    xk = nc.dram_tensor("xk", [E, s], BF16, kind="ExternalInput")
    xv = nc.dram_tensor("xv", [E, s], BF16, kind="ExternalInput")
    wq = nc.dram_tensor("wq", [E, 128], BF16, kind="ExternalInput")
    wk = nc.dram_tensor("wk", [E, 128], BF16, kind="ExternalInput")
    wv = nc.dram_tensor("wv", [E, 128], BF16, kind="ExternalInput")
    wo = nc.dram_tensor("wo", [128, E], BF16, kind="ExternalInput")
    bq2 = nc.dram_tensor("bq2", [128, 1], F32, kind="ExternalInput")
    bk2 = nc.dram_tensor("bk2", [128, 1], F32, kind="ExternalInput")
    outT = nc.dram_tensor("outT", [E, s], BF16, kind="ExternalOutput")

    def quad(dram_ap):
        # [512, c] dram slice -> [128, 4, c] AP (4 row-chunks stacked)
        return dram_ap.rearrange("(e p) c -> p e c", e=4)

    def sb4(sbuf_ap):
        # [128, 4*c] sbuf AP -> [128, 4, c] to pair with quad()
        return sbuf_ap.rearrange("p (e c) -> p e c", e=4)

    with TileContext(nc) as tc:
        with contextlib.ExitStack() as _stack:
            constp = _stack.enter_context(tc.tile_pool(name="const", bufs=1))
            if reps > 1:
                _stack.enter_context(tc.For_i(0, reps, 1))

            # ---- engine load balancing (ScalarE vs DVE) for PSUM work ----
            eng_load = {"A": 0.0, "D": 0.0}

            def pick_engine(a_cost, d_cost):
                if eng_load["A"] + a_cost <= eng_load["D"] + d_cost:
                    eng_load["A"] += a_cost
                    return "A"
                eng_load["D"] += d_cost
                return "D"

            def evac(dst, src, bias=None, scale=None, n=None):
                # PSUM -> SBUF move (opt. +bias or *scale) on the lighter engine
                if n is None:
                    n = src.shape[-1] if len(src.shape) == 2 else src.free_size()
                e = pick_engine(n * 0.833 + 330, n * 1.042 + 250)
                if e == "A":
                    nc.scalar.activation(
                        dst, src,
                        AF.Identity if bias is not None else AF.Copy,
                        bias=bias if bias is not None else 0.0,
                        scale=scale if scale is not None else 1.0,
                    )
                elif bias is not None:
                    nc.vector.tensor_scalar(
                        out=dst, in0=src, scalar1=bias, scalar2=None, op0=ADD
                    )
                elif scale is not None:
                    nc.vector.tensor_scalar(
                        out=dst, in0=src, scalar1=scale, scalar2=None, op0=MULT
                    )
                else:
                    nc.vector.tensor_copy(dst, src)

            # --- weights arrive pre-cast to bf16 from the host ---
            wbs = {}
            for name, dram in (("wq", wq), ("wk", wk), ("wv", wv)):
                rt = constp.tile([128, 512], BF16, tag=f"{name}_b", name=f"{name}_b")
                nc.sync.dma_start(sb4(rt[:]), quad(dram[:, :]))
                wbs[name] = rt
            wo_b = constp.tile([128, 512], BF16, tag="wo_b")
            nc.sync.dma_start(wo_b[:], wo[:, :])

            bq_t = constp.tile([128, 1], F32, tag="bq")
            nc.sync.dma_start(bq_t[:], bq2[:, :])
            bk_t = constp.tile([128, 1], F32, tag="bk")
            nc.sync.dma_start(bk_t[:], bk2[:, :])

            ident = constp.tile([128, 128], BF16, tag="ident")
            make_identity(nc, ident[:])

            # persistent activations, chunked for fine-grained deps
            qTs = [
                constp.tile([128, 512], F32R, tag=f"qT{i}", name=f"qT_{i}")
                for i in range(SC)
            ]
            kTs = [
                constp.tile([128, 512], F32R, tag=f"kT{i}", name=f"kT_{i}")
                for i in range(SC)
            ]
            attnTs = [
                constp.tile([128, 512], BF16, tag=f"attnT{i}", name=f"attnT_{i}")
                for i in range(SC)
            ]
            v2s = [
                constp.tile([128, 4 * 130], BF16, tag=f"v2_{i}", name=f"v2_{i}")
                for i in range(SC)
            ]
            for t in v2s:
                nc.gpsimd.memset(t[:], 1.0)

            pools = _stack.enter_context(contextlib.ExitStack())
            xinp = pools.enter_context(tc.tile_pool(name="xin", bufs=4))
            wtp = pools.enter_context(tc.tile_pool(name="wt", bufs=KC + 8))
            miscp = pools.enter_context(tc.tile_pool(name="miscp", bufs=8))
            ostp = pools.enter_context(tc.tile_pool(name="ostp", bufs=2))
            scoresp = pools.enter_context(
                tc.tile_pool(name="scoresp", bufs=3, space="PSUM")
            )
            attnp = pools.enter_context(tc.tile_pool(name="attnp", bufs=2, space="PSUM"))

            apsh = {}  # block -> [h0_tile, h1_tile]; each [128, 4*65] (one bank)

            def in_dma(dst, src):
                nc.sync.dma_start(dst, src)

            def emit_qk_proj(sc, which):
                dram, wname, btile, dst = (
                    (xq, "wq", bq_t, qTs[sc]) if which == "q" else (xk, "wk", bk_t, kTs[sc])
                )
                xt = xinp.tile([128, 2048], BF16, tag="xin", name=f"xt_{wname}_{sc}")
                in_dma(sb4(xt[:]), quad(dram[:, sc * 512 : (sc + 1) * 512]))
                ps = scoresp.tile([128, 512], F32, tag="sc", name=f"ps_{wname}_{sc}")
                for ec in range(4):
                    nc.tensor.matmul(
                        ps[:],
                        wbs[wname][:, ec * 128 : (ec + 1) * 128],
                        xt[:, ec * 512 : (ec + 1) * 512],
                        start=(ec == 0),
                        stop=(ec == 3),
                    )
                evac(dst[:], ps[:], bias=btile[:])

            def emit_v_proj(sc):
                xt = xinp.tile([128, 2048], BF16, tag="xin", name=f"xt_v_{sc}")
                in_dma(sb4(xt[:]), quad(xv[:, sc * 512 : (sc + 1) * 512]))
                v_ps = scoresp.tile([128, 512], F32, tag="sc", name=f"psv_{sc}")
                first = True
                for mc in range(4):
                    for ec in range(4):
                        nc.tensor.matmul(
                            v_ps[:, mc * 128 : (mc + 1) * 128],
                            xt[:, ec * 512 + mc * 128 : ec * 512 + (mc + 1) * 128],
                            wbs["wv"][:, ec * 128 : (ec + 1) * 128],
                            start=first,
                            stop=(mc == 3 and ec == 3),
                            skip_group_check=not first,
                        )
                        first = False
                # evac [128, (4 mc), 64] halves into the 65-strided v2 layout
                src3 = v_ps[:].rearrange("p (m z) -> p m z", m=4)
                dst3 = v2s[sc][:].rearrange("p (m z) -> p m z", m=4)
                for h in (0, 1):
                    evac(
                        dst3[:, :, h * 65 : h * 65 + 64],
                        src3[:, :, h * 64 : (h + 1) * 64],
                        n=256,
                    )

            wts = {}
            # DVE exp: single-instruction Schraudolph in bf16 bit-space.
            # i16 = round(x * 2^7/ln2 + (127*2^7 - c)) IS the bf16 bit pattern
            # of ~e^x (linear mantissa interp). c=7 zeroes the mean log-scale
            # mismatch vs the exact-exp tiles sharing softmax rows; residual
            # ripple ~1.8% rms on offloaded tiles only.
            SCHB_A = 184.6650171  # 2^7 / ln(2)
            SCHB_B = 127.0 * 128.0 - 7.0
            I16 = mybir.dt.int16

            def emit_scores_exp(sq, kc):
                # scores (both heads) -> one 2-bank psum region -> one exp
                ps = scoresp.tile([128, 1024], F32, tag="sc", name=f"ps_{sq}_{kc}")
                kslice = kTs[kc // 4][:, (kc % 4) * 128 : (kc % 4 + 1) * 128]
                for h in (0, 1):
                    nc.tensor.matmul(
                        ps[:, h * 512 : (h + 1) * 512],
                        kslice[h * 64 : (h + 1) * 64, :],
                        qTs[sq][h * 64 : (h + 1) * 64, :],
                        start=True,
                        stop=True,
                    )
                wt = wtp.tile([128, 1024], BF16, tag="wt", name=f"wt_{sq}_{kc}")
                e = pick_engine(1024 * 0.833 + 330, 1024 * 1.042 + 250)
                if e == "D":
                    nc.vector.tensor_scalar(
                        out=wt[:].bitcast(I16), in0=ps[:], scalar1=SCHB_A,
                        scalar2=SCHB_B, op0=MULT, op1=ADD,
                    )
                else:
                    nc.scalar.activation(wt[:], ps[:], AF.Exp)
                wts[(sq, kc)] = wt

            def emit_attn(sq, kc):
                wt = wts.pop((sq, kc))
                if sq not in apsh:
                    apsh[sq] = [
                        attnp.tile([128, 260], F32, tag="at", name=f"aps_{sq}_{h}")
                        for h in (0, 1)
                    ]
                for h in (0, 1):
                    for m in range(4):
                        # start=True clears has_written for the whole psum
                        # bank, so only the first matmul into each one-bank
                        # accumulator tile may set it; the other slices'
                        # first writes overwrite via cleared has_written.
                        nc.tensor.matmul(
                            apsh[sq][h][:, m * 65 : (m + 1) * 65],
                            wt[:, h * 512 + m * 128 : h * 512 + (m + 1) * 128],
                            v2s[kc // 4][
                                :,
                                (kc % 4) * 130 + h * 65 : (kc % 4) * 130 + (h + 1) * 65,
                            ],
                            start=(kc == 0 and m == 0),
                            stop=(kc == KC - 1 and m == 3),
                            skip_group_check=True,
                        )

            def emit_block_epilogue(sq):
                pairs = []
                for m in range(4):
                    pairs.append(
                        miscp.tile([128, 128], BF16, tag="pair", name=f"pair_{sq}_{m}")
                    )
                for h in (0, 1):
                    rcp = miscp.tile([128, 4], F32, tag="rcp", name=f"rcp_{sq}_{h}")
                    den = apsh[sq][h][:].rearrange("p (m z) -> p m z", m=4)
                    nc.vector.reciprocal(rcp[:], den[:, :, 64:65].squeeze(2))
                    for m in range(4):
                        evac(
                            pairs[m][:, h * 64 : (h + 1) * 64],
                            apsh[sq][h][:, m * 65 : m * 65 + 64],
                            scale=rcp[:, m : m + 1],
                        )
                del apsh[sq]
                ost = ostp.tile([128, 2048], BF16, tag="ost", name=f"ost_{sq}")
                for m in range(4):
                    tp = scoresp.tile([128, 128], BF16, tag="sc", name=f"tp_{sq}_{m}")
                    nc.tensor.transpose(tp[:], pairs[m][:], ident[:])
                    evac(attnTs[sq][:, m * 128 : (m + 1) * 128], tp[:])
                for oc in range(4):
                    po = scoresp.tile([128, 512], F32, tag="sc", name=f"po_{sq}_{oc}")
                    nc.tensor.matmul(
                        po[:],
                        wo_b[:, oc * 128 : (oc + 1) * 128],
                        attnTs[sq][:],
                        start=True,
                        stop=True,
                    )
                    evac(ost[:, oc * 512 : (oc + 1) * 512], po[:])
                nc.sync.dma_start(
                    quad(outT[:, sq * 512 : (sq + 1) * 512]), sb4(ost[:])
                )

            # The PE's 64x128-tiled score matmul pairs run CONCURRENTLY on
            # the two row-halves of the array (~229ns/pair measured), but
            # switching between 64-row tiling (scores) and full 128x128
            # mode (attn/projections) drains the array (~340ns/switch).
            # Scores are therefore emitted in batches so the mode switch
            # amortizes over 8+ pairs.

            # phase 1: k/v projections + q(0,1), interleaved with block-0
            # scores+attn and block-1 scores (lagged one chunk, merged into
            # the same 64-mode batch as block-0's)
            for sc in range(SC):
                emit_qk_proj(sc, "k")
                if sc <= 1:
                    emit_qk_proj(sc, "q")
                emit_v_proj(sc)
                for kc in range(4 * sc, 4 * sc + 4):
                    emit_scores_exp(0, kc)
                if sc >= 1 and SC > 1:
                    for kc in range(4 * (sc - 1), 4 * sc):
                        emit_scores_exp(1, kc)
                for kc in range(4 * sc, 4 * sc + 4):
                    emit_attn(0, kc)
            if SC > 1:
                for kc in range(4 * (SC - 1), KC):
                    emit_scores_exp(1, kc)
            # phase 2: per block, stay one block ahead on scores; epilogue
            # of the previous block runs up front so its apsh psum banks
            # free early for this block's accumulation. Scores and attn
            # alternate in batches of SBAT kc's to amortize mode switches.
            SBAT = 8
            for sq in range(1, SC):
                if sq + 1 < SC:
                    emit_qk_proj(sq + 1, "q")
                emit_block_epilogue(sq - 1)
                for u in range(0, KC, SBAT):
                    if sq + 1 < SC:
                        for kc in range(u, u + SBAT):
                            emit_scores_exp(sq + 1, kc)
                    for kc in range(u, u + SBAT):
                        emit_attn(sq, kc)
            emit_block_epilogue(SC - 1)
    nc._predicted_ns = getattr(tc, "predicted_ns", None)
    return _patch_to_json(nc)


# ---------------------------------------------------------------------------
# Host-side sharding / gathering
# ---------------------------------------------------------------------------


def make_in_maps(query, key_in, value, Wq, bq, Wk, bk, Wv, bv, Wo, bo, s=S):
    import ml_dtypes

    bf = ml_dtypes.bfloat16
    in_maps = []
    qT = [np.ascontiguousarray(query[b, :s].T).astype(bf) for b in range(B)]
    kT = [np.ascontiguousarray(key_in[b, :s].T).astype(bf) for b in range(B)]
    vT = [np.ascontiguousarray(value[b, :s].T).astype(bf) for b in range(B)]
    for c in range(N_CORES):
        b = c // 4
        hs = (c % 4) * 2 * DK  # column offset of this core's two heads
        in_maps.append(
            {
                "xq": qT[b],
                "xk": kT[b],
                "xv": vT[b],
                "wq": np.ascontiguousarray(Wq[:, hs : hs + 128]).astype(bf),
                "wk": np.ascontiguousarray(Wk[:, hs : hs + 128]).astype(bf),
                "wv": np.ascontiguousarray(Wv[:, hs : hs + 128]).astype(bf),
                "wo": np.ascontiguousarray(Wo[hs : hs + 128, :]).astype(bf),
                "bq2": np.ascontiguousarray(bq[hs : hs + 128, None]),
                "bk2": np.ascontiguousarray(bk[hs : hs + 128, None]),
            }
        )
    return in_maps


def assemble(results, bv, Wo, bo, s=S):
    out = np.zeros((B, s, E), np.float32)
    for c in range(N_CORES):
        out[c // 4] += results[c]["outT"].T.astype(np.float32)
    out += (bo + bv @ Wo)[None, None, :]
    return out


_nc_cache = {}


def kernel(query, key_in, value, Wq, bq, Wk, bk, Wv, bv, Wo, bo):
    _apply_patch()
    query = np.asarray(query, np.float32)
    key_in = np.asarray(key_in, np.float32)
    value = np.asarray(value, np.float32)
    Wq, bq = np.asarray(Wq, np.float32), np.asarray(bq, np.float32)
    Wk, bk = np.asarray(Wk, np.float32), np.asarray(bk, np.float32)
    Wv, bv = np.asarray(Wv, np.float32), np.asarray(bv, np.float32)
    Wo, bo = np.asarray(Wo, np.float32), np.asarray(bo, np.float32)

    if S not in _nc_cache:
        _nc_cache[S] = build(S)
    nc = _nc_cache[S]
    in_maps = make_in_maps(query, key_in, value, Wq, bq, Wk, bk, Wv, bv, Wo, bo)
    res = run_bass_kernel_spmd(nc, in_maps, core_ids=list(range(N_CORES)))
    return assemble(res.results, bv, Wo, bo)

